# revision 3
# baseline (speedup 1.0000x reference)
"""MoE routing kernel for Trainium2 (8 NeuronCores, SPMD data-parallel).

Problem: noisy top-k gating (B=4096 tokens, E=8 experts, K=2) + three
expert-combine heads: out_h = log(sum_e gates[:,e] * softmax(feat_h @ W_h[e])).

Sharding: pure data-parallel over the batch axis (512 tokens/core); the
small gating network + scalar load-balancing loss run on host (replicating
the reference op-for-op in fp32); each core computes the three heads for
its token shard with all 8 experts' weights replicated.

Device layout per core:
  x  [3, 128, 8, 512]  fp16  = per-head features, transposed: x[h, p, ks, t]
                               = feat_h[token=t, d=ks*128+p]
  w  [3, 4, 128, 8, 512] fp16 = weights packed in expert pairs:
                               w[h, pr, p, ks, j*256+c] = W_h[2*pr+j, ks*128+p, c]
  g  [128, 4, 8] fp32         = gates[t*128+p, e] -> g[p, t, e]
  y  [3, 4, 128, 256] fp32    = out_h[token chunk t] rows
Matmul: PSUM[tok 128, 512] += x_chunk[128d, 128tok].T @ w_pair[128d, 512]
accumulated over 8 d-chunks; two experts' logits per PSUM bank. Softmax
over the free dim (max on DVE, exp+row-sum fused on ACT), combine
acc += exp * (gate/sum) (scale on ACT, add on DVE), final log on ACT.
"""

import numpy as np

B = 4096
D = 1024
DP = 512
E = 8
C = 256
K = 2
H = 3
NCORES = 8
T = B // NCORES          # tokens per core = 512
P = 128
KS = D // P              # contraction chunks = 8
TC = T // P              # token chunks per core = 4
PAIRS = E // 2           # expert pairs = 4

NOISE_EPS = 1e-2
GATE_EPS = 1e-6

_NC_CACHE = {}


def _split_multiwait_drains(nc, max_waits=1):
    """This walrus build rejects instructions carrying more than one sync
    wait. Move extra waits onto preceding wait-only carrier instructions on
    the same engine (Drain clones for drains, EventSemaphore otherwise —
    the same carrier Tile's barriers use). Executing the waits in separate
    earlier instructions on the same engine is semantically identical."""
    import copy
    from concourse import mybir

    for f in nc.m.functions:
        for bb in f.blocks:
            new = []
            for inst in bb.instructions:
                w = inst.sync_info.on_wait if inst.sync_info else None
                if w and len(w) > max_waits:
                    for j, wi in enumerate(w[:-max_waits]):
                        if type(inst).__name__ == "InstDrain":
                            c = copy.deepcopy(inst)
                            c.name = f"{inst.name}_w{j}"
                            c.sync_info = copy.deepcopy(inst.sync_info)
                            c.sync_info.on_wait = [wi]
                            c.sync_info.on_update = []
                        else:
                            c = mybir.InstEventSemaphore(
                                name=f"{inst.name}_w{j}",
                                opcode="EventSemaphore",
                                engine=inst.engine,
                                ins=[],
                                outs=[],
                                sync_info=mybir.SyncInfo(on_wait=[wi], on_update=[]),
                            )
                        new.append(c)
                    inst.sync_info.on_wait = list(w[-max_waits:])
                new.append(inst)
            bb.instructions[:] = new
    return nc


def _build_nc(mm_dtype_name="float16"):
    import concourse.bass as bass
    import concourse.mybir as mybir
    import concourse.tile as tile

    f32 = mybir.dt.float32
    mmdt = getattr(mybir.dt, mm_dtype_name)
    AX = mybir.AxisListType.X
    ACTF = mybir.ActivationFunctionType

    nc = bass.Bass(target_bir_lowering=False, trn_type="TRN2")
    x = nc.declare_dram_parameter("x", [H, P, KS, T], mmdt, isOutput=False)
    w = nc.declare_dram_parameter("w", [H, PAIRS, P, KS, 2 * C], mmdt, isOutput=False)
    g = nc.declare_dram_parameter("g", [P, TC, E], f32, isOutput=False)
    y = nc.declare_dram_parameter("y", [H, TC, P, C], f32, isOutput=True)

    with tile.TileContext(nc) as tc:
        with (
            tc.tile_pool(name="xpool", bufs=1) as xpool,
            tc.tile_pool(name="wpool", bufs=2) as wpool,
            tc.tile_pool(name="gpool", bufs=1) as gpool,
            tc.tile_pool(name="ps", bufs=6, space="PSUM") as pspool,
            tc.tile_pool(name="ex", bufs=4) as expool,
            tc.tile_pool(name="acc", bufs=2) as accpool,
            tc.tile_pool(name="out", bufs=3) as outpool,
            tc.tile_pool(name="st", bufs=12) as stats,
        ):
            gt = gpool.tile([P, TC, E], f32)
            nc.sync.dma_start(out=gt, in_=g[:, :, :])

            x_tiles = []
            for h in range(H):
                xt = xpool.tile([P, KS, T], mmdt, tag=f"x{h}")
                nc.sync.dma_start(out=xt, in_=x[h])
                x_tiles.append(xt)

            for h in range(H):
                # All 4 expert-pair weight tiles for this head stay resident.
                w_tiles = []
                for pr in range(PAIRS):
                    wt = wpool.tile([P, KS, 2 * C], mmdt, tag=f"w{pr}")
                    nc.sync.dma_start(out=wt, in_=w[h, pr])
                    w_tiles.append(wt)

                for t in range(TC):
                    acc = accpool.tile([P, C], f32, tag="acc")
                    for pr in range(PAIRS):
                        ps = pspool.tile([P, 2 * C], f32, tag="psum")
                        for ks in range(KS):
                            nc.tensor.matmul(
                                ps,
                                lhsT=x_tiles[h][:, ks, P * t : P * (t + 1)],
                                rhs=w_tiles[pr][:, ks, :],
                                start=(ks == 0),
                                stop=(ks == KS - 1),
                            )
                        for j in range(2):
                            e = 2 * pr + j
                            sl = ps[:, C * j : C * (j + 1)]
                            negmx = stats.tile([P, 1], f32, tag="negmx")
                            nc.vector.reduce_max(negmx, sl, axis=AX, negate=True)
                            ex = expool.tile([P, C], f32, tag="ex")
                            s = stats.tile([P, 1], f32, tag="s")
                            nc.scalar.activation(
                                ex, sl, ACTF.Exp, bias=negmx, scale=1.0, accum_out=s
                            )
                            rc = stats.tile([P, 1], f32, tag="rc")
                            nc.vector.reciprocal(rc, s)
                            coef = stats.tile([P, 1], f32, tag="coef")
                            nc.vector.tensor_mul(
                                coef, rc, gt[:, t, e : e + 1]
                            )
                            if e == 0:
                                nc.vector.tensor_scalar_mul(acc, ex, coef)
                            else:
                                exs = expool.tile([P, C], f32, tag="exs")
                                nc.scalar.activation(
                                    exs, ex, ACTF.Copy, bias=0.0, scale=coef
                                )
                                nc.vector.tensor_add(acc, acc, exs)
                    ylog = outpool.tile([P, C], f32, tag="ylog")
                    nc.scalar.activation(ylog, acc, ACTF.Ln)
                    nc.sync.dma_start(out=y[h, t], in_=ylog)

    return _split_multiwait_drains(nc)


def _get_nc(mm_dtype_name="float16"):
    if mm_dtype_name not in _NC_CACHE:
        _NC_CACHE[mm_dtype_name] = _build_nc(mm_dtype_name)
    return _NC_CACHE[mm_dtype_name]


def _gating_host(sf, noise, w_gate, w_noise):
    """Replicates the reference noisy top-k gating op-for-op (fp32, CPU jax),
    returning (gates [B,E] f32, loss scalar f32)."""
    import jax
    import jax.numpy as jnp

    cpu = jax.devices("cpu")[0]

    def fn(sf, noise, w_gate, w_noise):
        clean_logits = sf @ w_gate
        noise_stddev = jax.nn.softplus(sf @ w_noise) + NOISE_EPS
        noisy_logits = clean_logits + noise * noise_stddev
        probs = jax.nn.softmax(noisy_logits, axis=1)
        m = min(K + 1, E)
        top_p, top_idx = jax.lax.top_k(probs, m)
        top_k_p = top_p[:, :K]
        top_k_idx = top_idx[:, :K]
        top_k_gates = top_k_p / (jnp.sum(top_k_p, axis=1, keepdims=True) + GATE_EPS)
        gates = (
            jnp.zeros_like(probs)
            .at[jnp.arange(sf.shape[0])[:, None], top_k_idx]
            .set(top_k_gates)
        )
        thr_in = top_p[:, K][:, None]
        thr_out = top_p[:, K - 1][:, None]
        is_in = noisy_logits > thr_in
        inv_sqrt2 = np.float32(1.0 / np.sqrt(2.0))
        prob_if_in = 0.5 * (1.0 + jax.lax.erf((clean_logits - thr_in) / noise_stddev * inv_sqrt2))
        prob_if_out = 0.5 * (1.0 + jax.lax.erf((clean_logits - thr_out) / noise_stddev * inv_sqrt2))
        load = jnp.sum(jnp.where(is_in, prob_if_in, prob_if_out), axis=0)
        importance = jnp.sum(gates, axis=0)

        def cv_squared(v):
            v = v.astype(jnp.float32)
            return jnp.var(v, ddof=1) / (jnp.mean(v) ** 2 + 1e-10)

        loss = (cv_squared(importance) + cv_squared(load)) * 1e-2
        return gates, loss

    with jax.default_device(cpu):
        gates, loss = fn(
            jnp.asarray(sf), jnp.asarray(noise), jnp.asarray(w_gate), jnp.asarray(w_noise)
        )
    return np.asarray(gates), np.asarray(loss)


def _pack_w(W, mmdt):
    # W [E, Din, C] -> [PAIRS, P, KS_w, 2C]; Din = KS_w*128
    ks_w = W.shape[1] // P
    return np.ascontiguousarray(
        W.reshape(PAIRS, 2, ks_w, P, C).transpose(0, 3, 2, 1, 4).reshape(PAIRS, P, ks_w, 2 * C)
    ).astype(mmdt)


def _pack_x(feat_shard, mmdt):
    # feat [T, Din] -> [P, KS_w, T]
    Tl, Din = feat_shard.shape
    return np.ascontiguousarray(
        feat_shard.T.reshape(Din // P, P, Tl).transpose(1, 0, 2)
    ).astype(mmdt)


def kernel(sf, of, ppf, pvf, noise, w_gate, w_noise, Ws, Wo, Wp):
    from concourse.bass_utils import run_bass_kernel_spmd

    sf = np.asarray(sf, dtype=np.float32)
    of = np.asarray(of, dtype=np.float32)
    ppf = np.asarray(ppf, dtype=np.float32)
    pvf = np.asarray(pvf, dtype=np.float32)
    noise = np.asarray(noise, dtype=np.float32)
    w_gate = np.asarray(w_gate, dtype=np.float32)
    w_noise = np.asarray(w_noise, dtype=np.float32)
    Ws = np.asarray(Ws, dtype=np.float32)
    Wo = np.asarray(Wo, dtype=np.float32)
    Wp = np.asarray(Wp, dtype=np.float32)

    gates, loss = _gating_host(sf, noise, w_gate, w_noise)

    mmdt = np.float16
    pf = np.concatenate([ppf, pvf], axis=1)  # [B, 1024]

    w_packed = np.stack([_pack_w(Ws, mmdt), _pack_w(Wo, mmdt), _pack_w(Wp, mmdt)])

    in_maps = []
    for c in range(NCORES):
        lo, hi = c * T, (c + 1) * T
        xh = np.stack(
            [_pack_x(sf[lo:hi], mmdt), _pack_x(of[lo:hi], mmdt), _pack_x(pf[lo:hi], mmdt)]
        )
        gh = np.ascontiguousarray(
            gates[lo:hi].reshape(TC, P, E).transpose(1, 0, 2)
        ).astype(np.float32)
        in_maps.append({"x": xh, "w": w_packed, "g": gh})

    nc = _get_nc("float16")
    res = run_bass_kernel_spmd(nc, in_maps, list(range(NCORES)))

    ys = np.empty((B, C), dtype=np.float32)
    yo = np.empty((B, C), dtype=np.float32)
    yp = np.empty((B, C), dtype=np.float32)
    for c in range(NCORES):
        yv = res.results[c]["y"]  # [H, TC, P, C]
        lo, hi = c * T, (c + 1) * T
        ys[lo:hi] = yv[0].reshape(T, C)
        yo[lo:hi] = yv[1].reshape(T, C)
        yp[lo:hi] = yv[2].reshape(T, C)

    return ys, yo, yp, loss


# revision 4
# speedup vs baseline: 122974.3295x; 122974.3295x over previous
"""MoE routing (noisy top-2 gating + 3 expert heads) on Trainium2, 8 NeuronCores.

Sharding: expert-parallel with host-side token dispatch, per the problem's
sharding hint. Each of the 8 cores owns one expert and processes only the
tokens routed to it (top-2 of 8 per token -> ~1024 of 4096 tokens/expert,
4x fewer FLOPs than the dense formulation). The host computes the small
noisy-top-k gating network (replicating the reference op-for-op in fp32 on
CPU jax), gathers each expert's tokens, and after the device pass combines
the two expert contributions per token (add + log). Each core computes, for
its token set and all three heads h:

    contrib[slot, c] = gate_e[slot] * softmax_c(feat_h[slot] @ W_h[e])

Matmuls run in fp16 (inputs ~N(0,1), logits ~N(0,1): fp16 keeps the logit
error ~4e-4 at full tensor-engine rate); accumulation, softmax and scaling
are fp32; contributions return as fp16 scaled by 1024 (keeps the smallest
softmax terms out of the fp16 subnormal range).

Device tensors per core (expert e), CAP = max tokens/expert rounded to 128:
  x [3, 128, 8, CAP] f16   gathered features, x[h][p, ks, s] = feat_h[tok(s), ks*128+p]
  w [3, 128, 8, 256] f16   w[h][p, ks, c] = W_h[e, ks*128+p, c]
  g [128, CAP/128]   f32   1024 * gate(tok(s), e), slot s=128*t+p at [p, t]
  y [3, CAP/128, 128, 256] f16  per-slot scaled contributions
"""

import numpy as np

B = 4096
D = 1024
E = 8
C = 256
K = 2
H = 3
NCORES = 8
P = 128
KS = D // P  # 8 contraction chunks of 128

NOISE_EPS = 1e-2
GATE_EPS = 1e-6
OUT_SCALE = 1024.0

_NC_CACHE = {}


def _split_multiwait_drains(nc, max_waits=1):
    """This walrus build rejects instructions carrying more than one sync
    wait. Move extra waits onto preceding wait-only carrier instructions on
    the same engine (Drain clones for drains, EventSemaphore otherwise —
    the same carrier Tile's barriers use). Executing the waits in separate
    earlier instructions on the same engine is semantically identical."""
    import copy
    from concourse import mybir

    for f in nc.m.functions:
        for bb in f.blocks:
            new = []
            for inst in bb.instructions:
                w = inst.sync_info.on_wait if inst.sync_info else None
                if w and len(w) > max_waits:
                    for j, wi in enumerate(w[:-max_waits]):
                        if type(inst).__name__ == "InstDrain":
                            c = copy.deepcopy(inst)
                            c.name = f"{inst.name}_w{j}"
                            c.sync_info = copy.deepcopy(inst.sync_info)
                            c.sync_info.on_wait = [wi]
                            c.sync_info.on_update = []
                        else:
                            c = mybir.InstEventSemaphore(
                                name=f"{inst.name}_w{j}",
                                opcode="EventSemaphore",
                                engine=inst.engine,
                                ins=[],
                                outs=[],
                                sync_info=mybir.SyncInfo(on_wait=[wi], on_update=[]),
                            )
                        new.append(c)
                    inst.sync_info.on_wait = list(w[-max_waits:])
                new.append(inst)
            bb.instructions[:] = new
    return nc


def _build_nc(cap):
    import concourse.bass as bass
    import concourse.mybir as mybir
    import concourse.tile as tile

    f32 = mybir.dt.float32
    f16 = mybir.dt.float16
    ACTF = mybir.ActivationFunctionType
    tc_n = cap // P

    nc = bass.Bass(target_bir_lowering=False, trn_type="TRN2")
    x = nc.declare_dram_parameter("x", [H, P, KS, cap], f16, isOutput=False)
    w = nc.declare_dram_parameter("w", [H, P, KS, C], f16, isOutput=False)
    g = nc.declare_dram_parameter("g", [P, tc_n], f32, isOutput=False)
    y = nc.declare_dram_parameter("y", [H, tc_n, P, C], f16, isOutput=True)

    with tile.TileContext(nc) as tc:
        with (
            tc.tile_pool(name="xpool", bufs=3) as xpool,
            tc.tile_pool(name="wpool", bufs=2) as wpool,
            tc.tile_pool(name="gpool", bufs=1) as gpool,
            tc.tile_pool(name="ps", bufs=4, space="PSUM") as pspool,
            tc.tile_pool(name="out", bufs=4) as outpool,
            tc.tile_pool(name="st", bufs=12) as stats,
        ):
            gt = gpool.tile([P, tc_n], f32)
            nc.sync.dma_start(out=gt, in_=g[:, :])

            for h in range(H):
                xt = xpool.tile([P, KS, cap], f16, tag="x")
                nc.sync.dma_start(out=xt, in_=x[h])
                wt = wpool.tile([P, KS, C], f16, tag="w")
                nc.sync.dma_start(out=wt, in_=w[h])

                for t in range(tc_n):
                    ps = pspool.tile([P, C], f32, tag="psum")
                    for ks in range(KS):
                        nc.tensor.matmul(
                            ps,
                            lhsT=xt[:, ks, P * t : P * (t + 1)],
                            rhs=wt[:, ks, :],
                            start=(ks == 0),
                            stop=(ks == KS - 1),
                        )
                    # No max-subtraction: |logits| <= ||feat|| * ||W col|| ~ 35,
                    # so fp32 exp cannot overflow and the row sum is exact
                    # enough; softmax = ex / sum(ex).
                    ex = outpool.tile([P, C], f32, tag="ex")
                    s = stats.tile([P, 1], f32, tag="s")
                    nc.scalar.activation(
                        ex, ps, ACTF.Exp, bias=0.0, scale=1.0, accum_out=s
                    )
                    rc = stats.tile([P, 1], f32, tag="rc")
                    nc.vector.reciprocal(rc, s)
                    coef = stats.tile([P, 1], f32, tag="coef")
                    nc.vector.tensor_mul(coef, rc, gt[:, t : t + 1])
                    yt = outpool.tile([P, C], f16, tag="yt")
                    nc.vector.tensor_scalar_mul(yt, ex, coef)
                    nc.sync.dma_start(out=y[h, t], in_=yt)

    return _split_multiwait_drains(nc)


def _get_nc(cap):
    if cap not in _NC_CACHE:
        _NC_CACHE[cap] = _build_nc(cap)
    return _NC_CACHE[cap]


def _gating_host(sf, noise, w_gate, w_noise):
    """Replicates the reference noisy top-k gating op-for-op (fp32 jax on
    CPU, eager, like the reference), returning (gates [B,E] f32, loss f32)."""
    import jax
    import jax.numpy as jnp

    cpu = jax.devices("cpu")[0]

    def fn(sf, noise, w_gate, w_noise):
        clean_logits = sf @ w_gate
        noise_stddev = jax.nn.softplus(sf @ w_noise) + NOISE_EPS
        noisy_logits = clean_logits + noise * noise_stddev
        probs = jax.nn.softmax(noisy_logits, axis=1)
        m = min(K + 1, E)
        top_p, top_idx = jax.lax.top_k(probs, m)
        top_k_p = top_p[:, :K]
        top_k_idx = top_idx[:, :K]
        top_k_gates = top_k_p / (jnp.sum(top_k_p, axis=1, keepdims=True) + GATE_EPS)
        gates = (
            jnp.zeros_like(probs)
            .at[jnp.arange(sf.shape[0])[:, None], top_k_idx]
            .set(top_k_gates)
        )
        thr_in = top_p[:, K][:, None]
        thr_out = top_p[:, K - 1][:, None]
        is_in = noisy_logits > thr_in
        inv_sqrt2 = np.float32(1.0 / np.sqrt(2.0))
        prob_if_in = 0.5 * (
            1.0 + jax.lax.erf((clean_logits - thr_in) / noise_stddev * inv_sqrt2)
        )
        prob_if_out = 0.5 * (
            1.0 + jax.lax.erf((clean_logits - thr_out) / noise_stddev * inv_sqrt2)
        )
        load = jnp.sum(jnp.where(is_in, prob_if_in, prob_if_out), axis=0)
        importance = jnp.sum(gates, axis=0)

        def cv_squared(v):
            v = v.astype(jnp.float32)
            return jnp.var(v, ddof=1) / (jnp.mean(v) ** 2 + 1e-10)

        loss = (cv_squared(importance) + cv_squared(load)) * 1e-2
        return gates, loss

    with jax.default_device(cpu):
        gates, loss = fn(
            jnp.asarray(sf), jnp.asarray(noise), jnp.asarray(w_gate), jnp.asarray(w_noise)
        )
    return np.asarray(gates), np.asarray(loss)


def _dispatch(gates):
    """Token -> expert routing from the dense gate matrix.
    Returns (top2 [B,2] expert ids, slot [B,2] slot indices, token_lists)."""
    n = gates.shape[0]
    top2 = np.argsort(-gates, axis=1, kind="stable")[:, :2].astype(np.int64)
    token_lists = []
    slot = np.zeros((n, 2), dtype=np.int64)
    for e in range(E):
        mask = top2 == e
        toks = np.nonzero(mask.any(axis=1))[0]
        token_lists.append(toks)
        pos = np.arange(len(toks))
        which = mask[toks]  # [cnt, 2]
        slot[toks[which[:, 0]], 0] = pos[which[:, 0]]
        slot[toks[which[:, 1]], 1] = pos[which[:, 1]]
    return top2, slot, token_lists


def kernel(sf, of, ppf, pvf, noise, w_gate, w_noise, Ws, Wo, Wp):
    from concourse.bass_utils import run_bass_kernel_spmd

    sf = np.asarray(sf, dtype=np.float32)
    of = np.asarray(of, dtype=np.float32)
    ppf = np.asarray(ppf, dtype=np.float32)
    pvf = np.asarray(pvf, dtype=np.float32)
    noise = np.asarray(noise, dtype=np.float32)
    w_gate = np.asarray(w_gate, dtype=np.float32)
    w_noise = np.asarray(w_noise, dtype=np.float32)
    Ws = np.asarray(Ws, dtype=np.float32)
    Wo = np.asarray(Wo, dtype=np.float32)
    Wp = np.asarray(Wp, dtype=np.float32)

    gates, loss = _gating_host(sf, noise, w_gate, w_noise)
    top2, slot, token_lists = _dispatch(gates)

    cap = max(len(t) for t in token_lists)
    cap = ((cap + P - 1) // P) * P
    tc_n = cap // P

    mmdt = np.float16
    pf = np.concatenate([ppf, pvf], axis=1)
    feats = [sf, of, pf]

    in_maps = []
    for e in range(E):
        toks = token_lists[e]
        cnt = len(toks)
        xh = np.zeros((H, P, KS, cap), dtype=mmdt)
        for h in range(H):
            fg = feats[h][toks]  # [cnt, 1024]
            xh[h, :, :, :cnt] = fg.T.reshape(KS, P, cnt).transpose(1, 0, 2).astype(mmdt)
        wh = np.stack(
            [
                np.ascontiguousarray(W[e].reshape(KS, P, C).transpose(1, 0, 2)).astype(mmdt)
                for W in (Ws, Wo, Wp)
            ]
        )
        # slot s = 128*t + p lives at gh[p, t]; OUT_SCALE pre-scaling keeps
        # the f16 output contributions out of the subnormal range
        flat = np.zeros(cap, dtype=np.float32)
        flat[:cnt] = gates[toks, e] * np.float32(OUT_SCALE)
        gh = np.ascontiguousarray(flat.reshape(tc_n, P).T)
        in_maps.append({"x": xh, "w": wh, "g": gh})

    nc = _get_nc(cap)
    try:
        res = run_bass_kernel_spmd(nc, in_maps, list(range(NCORES)))
    except Exception:
        # transient device wedges (e.g. NRT unrecoverable after an aborted
        # run) clear on retry
        res = run_bass_kernel_spmd(nc, in_maps, list(range(NCORES)))

    # combine on host: out[b] = contrib[e1][slot1] + contrib[e2][slot2], then log
    Y = np.stack(
        [res.results[e]["y"].reshape(H, cap, C).astype(np.float32) for e in range(E)]
    )
    e1, e2 = top2[:, 0], top2[:, 1]
    s1, s2 = slot[:, 0], slot[:, 1]
    log_scale = np.float32(np.log(OUT_SCALE))
    outs = []
    for h in range(H):
        comb = Y[e1, h, s1] + Y[e2, h, s2]
        outs.append(np.log(comb) - log_scale)
    ys, yo, yp = outs
    return ys, yo, yp, loss
